# revision 64
# baseline (speedup 1.0000x reference)
"""Multi-head attention (b=2, n=2048, d=1024, H=16 heads) on 8 TRN2 NeuronCores.

Sharding: core c = (b, g) with b = c // 4 (data parallel over batch) and
g = c % 4 (tensor parallel over head groups of 4 heads).  Each core computes
qkv projections for its 4 heads, full softmax attention for those heads, and
a partial output projection y_partial = A_heads @ w_out[g*256:(g+1)*256].
The host sums the 4 partials per batch and adds b_out.

v2 schedule (from trace analysis of the 214us baseline):
  - ScalarE exp is the binding engine (128 x 1114ns ACTIVATE); runtime =
    boot + gap-free exp stream + tail, so the whole design minimizes the
    three non-exp components.
  - boot: input DMA was ISSUE-bound (34 dma_starts x ~650ns serialized on
    the sync queue).  Now ~9 coalesced issues split across the sync AND
    scalar queues (both are HWDGE engines; transfers stripe over all 16
    DMA engines regardless).  v-projection for chunks 0-1 and the kq
    prefix fill the PE during the DMA window.
  - pr-major block order (0,0),(1,0),(2,0),(3,0),(0,1).. so kt m1 is not
    needed until step 64 (c-major needed ALL of kt by step 16, overloading
    block 0 by ~6us).  JIT kq groups spread at 2 matmuls/step via a greedy
    per-step load balancer (4-matmul bursts overflowed the ~360ns/step PE
    slack and stalled exp by ~600ns once per block).
  - norm of block B runs during B+1 at t=4/6 (not 12/14): with pr-major
    there is no DVE congestion early in a block, and the earlier norm
    opens a 10-step window for the out-proj units of the previous chunk.
  - out-proj units are atomic (ks0+ks1 back to back) gated on both halves
    of their at-chunk; pinned behind the attention stream as before.
  - tail: the last block (3,1) is split into two 256-col segments A/B.
    A's epilogue (denominator DMA-transpose roundtrip, norm, proj m12/13,
    casts, y writes) hides behind B's 16-step exp stream; only B's own
    epilogue (~512 denominators) plus proj m14/15 remain exposed after the
    last exp.  Fast path for the tail denominators: ScalarE (idle after
    the last exp) copies the PSUM denominator rows out immediately, the
    gather/scatter DMAs issue from scalar+sync+gpsimd queues in parallel,
    and const matmuls keep the PE p-state up through the reciprocal
    roundtrip.
Matmuls run in bf16 (fp32 PSUM accumulation), y written bf16.
"""

import os
import sys

for _p in ("/opt/trn_rl_repo",):
    if _p not in sys.path and os.path.isdir(_p):
        sys.path.insert(0, _p)

import ml_dtypes
import numpy as np

import concourse.bass as bass
import concourse.mybir as mybir
import concourse.tile as tile
from concourse import bacc

P = 128
D = 1024          # model dim
N = 2048          # sequence length
HD = 64           # head dim
GH = 4            # heads per core
DG = GH * HD      # 256 projected cols per core
KD = D // P       # 8 k-tiles over model dim
NT = N // P       # 16 tiles over sequence
QC = 512          # n_q chunk size
NQC = N // QC     # 4 chunks
SCALE = HD ** -0.5

F32 = mybir.dt.float32
BF16 = mybir.dt.bfloat16

Exp = mybir.ActivationFunctionType.Exp

# segments: (chunk, pr, q0, q1) — 7 full blocks pr-major, last block split
# into two 256-col halves so its epilogue pipelines with compute.
SEGS = [(0, 0, 0, QC), (1, 0, 0, QC), (2, 0, 0, QC), (3, 0, 0, QC),
        (0, 1, 0, QC), (1, 1, 0, QC), (2, 1, 0, QC),
        (3, 1, 0, 256), (3, 1, 256, QC)]
NSEG = len(SEGS)
NSTEP = 16 * NSEG  # 144


def build_nc():
    nc = bacc.Bacc("TRN2")

    # all inputs pre-swizzled on host to [128 partitions, ...contiguous]
    xt = nc.declare_dram_parameter("xt", [P, NQC, KD, QC], BF16, isOutput=False)
    wq = nc.declare_dram_parameter("wq", [P, KD, DG], BF16, isOutput=False)
    wk = nc.declare_dram_parameter("wk", [P, KD, DG], BF16, isOutput=False)
    wv = nc.declare_dram_parameter("wv", [P, KD, DG], BF16, isOutput=False)
    wo = nc.declare_dram_parameter("wo", [P, 2, D], BF16, isOutput=False)
    # wo rows 192-255 (ks=1 upper half) pre-shifted to partitions 0-63: the
    # tail projection contracts them against un-hopped half-1 numerators
    wt = nc.declare_dram_parameter("wt", [HD, D], BF16, isOutput=False)
    y = nc.declare_dram_parameter("y", [N, D], BF16, isOutput=True)

    y_r = y[:, :].rearrange("(o p) n -> p o n", p=P)      # [128, 16, 1024]

    with tile.TileContext(nc) as tc, nc.allow_low_precision("bf16 attention"):
        with (
            tc.tile_pool(name="wpool", bufs=1) as wpool,
            tc.tile_pool(name="qkvpool", bufs=1) as qkvpool,
            tc.tile_pool(name="attnpool", bufs=1) as attnpool,
            tc.tile_pool(name="xpool", bufs=1) as xpool,
            tc.tile_pool(name="work", bufs=6) as work,
            tc.tile_pool(name="epi", bufs=2) as epi,
            tc.tile_pool(name="outp", bufs=8) as outp,
            tc.tile_pool(name="ps_a", bufs=2, space="PSUM") as ps_a,
            tc.tile_pool(name="ps_st", bufs=2, space="PSUM") as ps_st,
            tc.tile_pool(name="ps_o", bufs=2, space="PSUM") as ps_o,
        ):
            # --- persistent SBUF tiles ---
            wk_sb = wpool.tile([P, KD, DG], BF16, tag="wk")
            wq_sb = wpool.tile([P, KD, DG], BF16, tag="wq")
            wv_sb = wpool.tile([P, KD, DG], BF16, tag="wv")
            wo_sb = wpool.tile([P, 2, D], BF16, tag="wo")
            xt_sb = xpool.tile([P, NQC, KD, QC], BF16, tag="xt")

            wt_sb = wpool.tile([HD, D], BF16, tag="wt")
            qt_sb = qkvpool.tile([P, 2, N], BF16, tag="qt")   # [256, 2048] qT
            kt_sb = qkvpool.tile([P, 2, N], BF16, tag="kt")   # [256, 2048] kT
            vg_sb = qkvpool.tile([P, NT, GH, 66], BF16, tag="vg")  # v + ones
            at_sb = attnpool.tile([P, 2, N], BF16, tag="at")  # attn_outT

            # --- coalesced input DMA, issues split over sync+scalar queues
            # (each dma_start costs ~650ns of issue time on its queue and
            # its transfer stripes across all 16 DMA engines; the baseline's
            # 34 issues serialized ~20us on the sync queue alone) ---
            # --- wave-1 DMA: just the critical set for the first scores
            # (transfers on one queue run near-serially at ~105GB/s each;
            # two HWDGE queues give ~2x.  Anything queued alongside the
            # critical set delays it 1:1, so the rest waits in wave 2.)
            nc.sync.dma_start(xt_sb[:, 0, 0:4], xt[:, 0, 0:4])
            nc.scalar.dma_start(xt_sb[:, 0, 4:8], xt[:, 0, 4:8])
            nc.sync.dma_start(wk_sb[:, 0:4], wk[:, 0:4])
            nc.scalar.dma_start(wq_sb[:, 0:4], wq[:, 0:4])
            nc.sync.dma_start(wk_sb[:, 4:8], wk[:, 4:8])
            nc.scalar.dma_start(wq_sb[:, 4:8], wq[:, 4:8])
            nc.sync.dma_start(wv_sb[:, 0:4], wv[:, 0:4])
            nc.scalar.dma_start(wv_sb[:, 4:8], wv[:, 4:8])

            nc.scalar.copy(
                vg_sb[:, :, :, HD:], nc.const_aps.tensor(1.0, (P, NT, GH, 2), F32)
            )
            ones_t = wpool.tile([P, HD], BF16, tag="ones_t")
            nc.scalar.copy(
                ones_t[HD:HD + 1, :], nc.const_aps.tensor(1.0, (1, HD), F32)
            )

            # PE p-state warmup while the wave-1 DMAs land (~80 x ~80ns
            # covers the window to data arrival without overrunning it)
            warm_w = nc.const_aps.tensor(1.0, (P, P), BF16)
            warm_ps = ps_st.tile([P, 2, QC], F32, tag="st", name="warm")
            for _ in range(120):
                nc.tensor.matmul(
                    warm_ps[:, 0, 0:P], warm_w, warm_w, start=True, stop=True
                )

            # ---------------- emitters ----------------
            def emit_kq_piece(which, w_sb, dst, m, c, ks, state):
                # resumable slice of an 8-matmul k/q projection group
                if state.get("ps") is None:
                    state["ps"] = ps_a.tile(
                        [P, QC], F32, tag="a", name=f"{which}ps_{m}_{c}"
                    )
                ps = state["ps"]
                mm = None
                for k in ks:
                    mm = nc.tensor.matmul(
                        ps[:],
                        w_sb[:, k, m * P:(m + 1) * P],
                        xt_sb[:, c, k, :],
                        start=(k == 0),
                        stop=(k == KD - 1),
                    )
                if ks[-1] == KD - 1:
                    nc.vector.tensor_copy(dst[:, m, c * QC:(c + 1) * QC], ps[:])
                return mm

            def emit_v_piece(t, ks, after=None):
                c4, r4 = t // 4, t % 4
                ps = v_state.get(t)
                if ps is None:
                    ps = ps_a.tile([P, QC], F32, tag="a", name=f"vps_{t}")
                    v_state[t] = ps
                mm = None
                for k in ks:
                    mm = nc.tensor.matmul(
                        ps[:, :DG],
                        xt_sb[:, c4, k, r4 * P:(r4 + 1) * P],
                        wv_sb[:, k, :],
                        start=(k == 0),
                        stop=(k == KD - 1),
                    )
                    if after is not None:
                        bass._add_dep_helper(mm.ins, after.ins, sync=False,
                                             reason="order after scores")
                        after = None
                if ks[-1] == KD - 1:
                    nc.vector.tensor_copy(
                        vg_sb[:, t, :, 0:HD],
                        ps[:, :DG].rearrange("p (h e) -> p h e", h=GH),
                    )
                return mm

            v_state = {}

            last_scores = {}

            def emit_scores(seg, t):
                c, pr, q0, q1 = SEGS[seg]
                cs = slice(c * QC + q0, c * QC + q1)
                ts_ = slice(t * P, (t + 1) * P)
                st = ps_st.tile([P, 2, QC], F32, tag="st", name=f"st_{seg}_{t}")
                for half in range(2):
                    hs = slice(half * HD, (half + 1) * HD)
                    last_scores["mm"] = nc.tensor.matmul(
                        st[:, half, 0:q1 - q0],
                        kt_sb[hs, pr, ts_],
                        qt_sb[hs, pr, cs],
                        start=True,
                        stop=True,
                    )
                return st

            def emit_exp(seg, t, st):
                c, pr, q0, q1 = SEGS[seg]
                w = q1 - q0
                e = work.tile([P, 2, w], BF16, tag=f"exp{w}", name=f"e_{seg}_{t}")
                nc.scalar.activation(e[:], st[:, :, 0:w], Exp, scale=SCALE)
                return e

            def emit_pv(seg, t, e, o_ps):
                c, pr, q0, q1 = SEGS[seg]
                last = None
                for half in range(2):
                    h = 2 * pr + half
                    last = nc.tensor.matmul(
                        o_ps[half][:, 0:q1 - q0],
                        vg_sb[:, t, h, 0:HD + 1],
                        e[:, half, :],
                        start=(t == 0),
                        stop=(t == NT - 1),
                    )
                return last

            def alloc_o(seg):
                o_ps = []
                for half in range(2):
                    o_full = ps_o.tile(
                        [P, QC], F32, tag="o", name=f"o_{seg}_{half}"
                    )
                    o_ps.append(o_full[: HD + 1])
                return o_ps

            def emit_epi_stage(seg, o_ps, tail=False):
                # stage PV accumulators to SBUF; invert the denominators
                # TRANSPOSED via a DMA gather to [128, w//64] (1024 denoms on
                # one partition would cost ~7ns/elem on the DVE), reciprocal,
                # scatter back to partition 0 for the gpsimd broadcast.
                c, pr, q0, q1 = SEGS[seg]
                w = q1 - q0
                i8 = w // 64
                o_sb = epi.tile([HD + 1, 2, w], F32, tag=f"osb{w}",
                                name=f"osb_{seg}")
                dT = epi.tile([P, i8], F32, tag=f"dT{i8}", name=f"dT_{seg}")
                if tail:
                    # ScalarE is idle after the last exp: pull the denom rows
                    # out fast so the gather doesn't wait for the full o_sb
                    # copies.  Wide-row gather/scatter shapes ([w//32, 32]
                    # instead of [*, w//64]) cut the DMA descriptor count
                    # ~8x, and the three DMAs issue on three queues.
                    rows = w // 32
                    den = epi.tile([HD + 1, 2, w], F32, tag=f"den{w}",
                                   name=f"den_{seg}")
                    dTt = epi.tile([2 * rows, 32], F32, tag="dTt",
                                   name=f"dTt_{seg}")
                    for half in range(2):
                        nc.scalar.copy(
                            den[HD:HD + 1, half, :], o_ps[half][HD:HD + 1, 0:w]
                        )
                    nc.scalar.dma_start(
                        dTt[0:rows, :],
                        den[HD:HD + 1, 0, :].rearrange("o (b i) -> o b i", i=32),
                    )
                    nc.sync.dma_start(
                        dTt[rows:2 * rows, :],
                        den[HD:HD + 1, 1, :].rearrange("o (b i) -> o b i", i=32),
                    )
                    rcTt = epi.tile([2 * rows, 32], F32, tag="rcTt",
                                    name=f"rcTt_{seg}")
                    nc.vector.reciprocal(rcTt[:], dTt[:, :])
                    rc = epi.tile([1, 2, w], F32, tag=f"rc{w}",
                                  name=f"rct_{seg}")
                    nc.gpsimd.dma_start(
                        rc[0:1, :, :].rearrange("o h (b i) -> o (h b) i", i=32),
                        rcTt[:, :],
                    )
                    for half in range(2):
                        nc.vector.tensor_copy(o_sb[:, half, :],
                                              o_ps[half][:, 0:w])
                    return o_sb, rc
                else:
                    for half in range(2):
                        nc.vector.tensor_copy(o_sb[:, half, :],
                                              o_ps[half][:, 0:w])
                        nc.sync.dma_start(
                            dT[HD * half:HD * (half + 1), :],
                            o_sb[HD:HD + 1, half, :].rearrange(
                                "o (b i) -> o b i", i=i8
                            ),
                        )
                rcT = epi.tile([P, i8], F32, tag=f"rcT{i8}", name=f"rcT_{seg}")
                nc.vector.reciprocal(rcT[:], dT[:, :])
                rc = epi.tile([1, 2, w], F32, tag=f"rc{w}", name=f"rc_{seg}")
                nc.sync.dma_start(
                    rc[0:1, :, :].rearrange("o h (b i) -> o (h b) i", i=i8),
                    rcT[:, :],
                )
                return o_sb, rc

            def emit_epi_norm(seg, o_sb, rc, s, pieces):
                # one piece of the normalization: A^T = o[:64]*(1/o[64]).
                c, pr, q0, q1 = SEGS[seg]
                wseg = q1 - q0
                rbs = epi.tile([HD, 2, wseg], F32, tag=f"rbs{wseg}",
                               name=f"rbs_{seg}", uniquify=True)
                w = wseg // pieces
                ss = slice(s * w, (s + 1) * w)
                a0 = c * QC + q0 + s * w
                nc.gpsimd.partition_broadcast(
                    rbs[:, :, ss], rc[0:1, :, ss], channels=HD
                )
                # half 1 first: its at-write goes through an extra SBUF->SBUF
                # DMA hop (partition shift), so start that chain earliest
                stg = work.tile(
                    [HD, w], BF16, tag=f"stg{w}", name=f"stg_{seg}_{s}"
                )
                nc.vector.tensor_mul(stg[:], o_sb[0:HD, 1, ss], rbs[:, 1, ss])
                nc.sync.dma_start(at_sb[HD:P, pr, a0:a0 + w], stg[:])
                nc.vector.tensor_mul(
                    at_sb[0:HD, pr, a0:a0 + w],
                    o_sb[0:HD, 0, ss],
                    rbs[:, 0, ss],
                )

            # seg 7 (tail-A): half-1 numerators stay on partitions 0-63 in
            # stg7 (no SBUF->SBUF partition-shift DMA: those land on a slow
            # table queue whose drain gates kernel completion), and the
            # m12/13 proj contracts them against the pre-shifted wt copy
            stg7 = epi.tile([HD, 256], BF16, tag="stg7", name="stg7")

            def emit_norm_nohop(seg, o_sb, rc, s):
                c, pr, q0, q1 = SEGS[seg]
                w = (q1 - q0) // 2
                rbs = epi.tile([HD, 2, q1 - q0], F32, tag="rbsA",
                               name=f"rbsA_{seg}", uniquify=True)
                ss = slice(s * w, (s + 1) * w)
                a0 = c * QC + q0 + s * w
                nc.gpsimd.partition_broadcast(
                    rbs[:, :, ss], rc[0:1, :, ss], channels=HD
                )
                nc.vector.tensor_mul(stg7[:, ss], o_sb[0:HD, 1, ss],
                                     rbs[:, 1, ss])
                nc.vector.tensor_mul(
                    at_sb[0:HD, pr, a0:a0 + w], o_sb[0:HD, 0, ss],
                    rbs[:, 0, ss],
                )

            def emit_proj_split(m, nn, gate, stg, soff, y_q=None):
                # proj unit with the ks=1 contraction split K=64+K=64 so the
                # half-1 numerators are read from stg (partitions 0-63)
                ps = ps_a.tile([P, QC], F32, tag="a", name=f"yps_{m}_{nn}")
                mm = nc.tensor.matmul(
                    ps[:], at_sb[:, 0, m * P:(m + 1) * P],
                    wo_sb[:, 0, nn * QC:(nn + 1) * QC], start=True, stop=False,
                )
                if gate is not None:
                    bass._add_dep_helper(mm.ins, gate.ins, sync=False,
                                         reason="defer proj")
                nc.tensor.matmul(
                    ps[:], at_sb[0:HD, 1, m * P:(m + 1) * P],
                    wo_sb[0:HD, 1, nn * QC:(nn + 1) * QC],
                    start=False, stop=False,
                )
                nc.tensor.matmul(
                    ps[:], stg[:, soff:soff + P],
                    wt_sb[:, nn * QC:(nn + 1) * QC], start=False, stop=True,
                )
                ysb = outp.tile([P, QC], BF16, tag="y", name=f"y_{m}_{nn}")
                nc.vector.tensor_copy(ysb[:], ps[:])
                (y_q or nc.sync).dma_start(
                    y_r[:, m, nn * QC:(nn + 1) * QC], ysb[:]
                )

            def emit_proj_unit(m, nn, gate, cast_eng=None, y_q=None):
                # atomic out-proj unit: both ks matmuls, drain cast, y write.
                # order-pinned behind the attention stream (the sim
                # undercosts the DVE reciprocal chain feeding at_sb, and an
                # optimistically-early proj in the static PE queue
                # head-of-line blocks the scores on hardware)
                ps = ps_a.tile([P, QC], F32, tag="a", name=f"yps_{m}_{nn}")
                for ks in range(2):
                    mm = nc.tensor.matmul(
                        ps[:],
                        at_sb[:, ks, m * P:(m + 1) * P],
                        wo_sb[:, ks, nn * QC:(nn + 1) * QC],
                        start=(ks == 0),
                        stop=(ks == 1),
                    )
                    if gate is not None:
                        bass._add_dep_helper(mm.ins, gate.ins, sync=False,
                                             reason="defer proj")
                ysb = outp.tile([P, QC], BF16, tag="y", name=f"y_{m}_{nn}")
                if cast_eng == "scalar":
                    nc.scalar.copy(ysb[:], ps[:])
                else:
                    nc.vector.tensor_copy(ysb[:], ps[:])
                (y_q or nc.sync).dma_start(
                    y_r[:, m, nn * QC:(nn + 1) * QC], ysb[:]
                )

            # ---------------- extras load balancer ----------------
            # load[u] = (base PE cost) - (ACT budget): minimizing load
            # equalizes *overflow*, so extras prefer steps with headroom
            # (the 256-wide tail segments have short exps = little slack).
            load = [0.0] * NSTEP
            for u in range(NSTEP):
                _, _, q0, q1 = SEGS[u // 16]
                w = q1 - q0
                load[u] = (220.0 + 2 * w * 0.42 + 80.0) - (172 + 2 * w) / 1.2
            extras = {u: [] for u in range(NSTEP)}

            def place(cost, est, lst, fn):
                best = min(range(est, lst + 1), key=lambda u: load[u])
                load[best] += cost
                extras[best].append(fn)
                return best

            def sched_group(cost, est, lst, fns):
                # pieces of one group on CONSECUTIVE steps (the group holds
                # a ps_a buf from first piece to last; adjacency bounds the
                # hold time so the 2-buf pool doesn't starve)
                n = len(fns)
                b0 = min(range(est, lst - n + 2),
                         key=lambda u: max(load[u + i] for i in range(n)))
                for i, fn in enumerate(fns):
                    load[b0 + i] += cost
                    extras[b0 + i].append(fn)

            # v tiles 1-15 inside block 0's steps so they interleave with the
            # scores/PV stream instead of sitting as one blob in the PE FIFO
            # (tile 0 runs in the boot, pinned behind the first scores).
            # est follows each chunk's wave-2 DMA arrival — work placed in
            # the FIFO before its data exists head-of-line blocks the PE.
            V_EST = {0: 0, 1: 2, 2: 4, 3: 6}   # per xt chunk
            for t in (1, 2):
                place(880, t - 1, t - 1,
                      lambda gate, tt=t: emit_v_piece(tt, list(range(KD))))
            for t in range(3, NT):
                fns = [
                    lambda gate, tt=t: emit_v_piece(tt, [0, 1, 2, 3]),
                    lambda gate, tt=t: emit_v_piece(tt, [4, 5, 6, 7]),
                ]
                sched_group(440, min(V_EST[t // 4], t - 2), t - 1, fns)
            # kt m0 c1-3 feed block 0's own scores at t=4/8/12: emitted
            # before those scores but not before their xt chunk arrives
            for c, est, lst in ((1, 1, 2), (2, 4, 6), (3, 6, 10)):
                st_c = {}
                sched_group(880, est, lst, [
                    lambda gate, cc=c, st=st_c: emit_kq_piece(
                        "k", wk_sb, kt_sb, 0, cc, [0, 1, 2, 3], st),
                    lambda gate, cc=c, st=st_c: emit_kq_piece(
                        "k", wk_sb, kt_sb, 0, cc, [4, 5, 6, 7], st),
                ])

            # JIT kq groups at 2-matmul granularity.  kq group (which, m, c)
            # produces keys/queries consumed from step `need` onward.
            def kq_fns(which, w_sb, dst, m, c):
                state = {}
                return [
                    lambda gate, st=state, kk=k: emit_kq_piece(
                        which, w_sb, dst, m, c, [kk, kk + 1], st
                    )
                    for k in (0, 2, 4, 6)
                ]

            sched_group(440, 3, 14, kq_fns("q", wq_sb, qt_sb, 0, 1))
            sched_group(440, 16, 30, kq_fns("q", wq_sb, qt_sb, 0, 2))
            sched_group(440, 32, 46, kq_fns("q", wq_sb, qt_sb, 0, 3))
            sched_group(440, 48, 62, kq_fns("q", wq_sb, qt_sb, 1, 0))
            sched_group(440, 64, 78, kq_fns("q", wq_sb, qt_sb, 1, 1))
            sched_group(440, 80, 94, kq_fns("q", wq_sb, qt_sb, 1, 2))
            sched_group(440, 96, 110, kq_fns("q", wq_sb, qt_sb, 1, 3))
            # kt m1: needed from step 64 (seg 4 = first pr=1 block); c3 keys
            # (1536+) first touched at seg4 t12 = step 76
            sched_group(440, 16, 40, kq_fns("k", wk_sb, kt_sb, 1, 0))
            sched_group(440, 24, 52, kq_fns("k", wk_sb, kt_sb, 1, 1))
            sched_group(440, 40, 60, kq_fns("k", wk_sb, kt_sb, 1, 2))
            sched_group(440, 48, 70, kq_fns("k", wk_sb, kt_sb, 1, 3))

            # out-proj: chunk c's at is complete once norm of its pr=1 block
            # (seg 4+c) lands during seg 5+c at t=4/6 -> window opens t8.
            # chunk 3: m12/13 open after seg7(A)'s norm early in seg8;
            # m14/15 run in the tail.
            proj_windows = {0: (88, 119), 1: (104, 131), 2: (120, 139)}
            for c in range(3):
                est, lst = proj_windows[c]
                for j in range(8):
                    m, nn = 4 * c + j // 2, j % 2
                    place(440, est, lst,
                          lambda g, mm=m, nn2=nn: emit_proj_unit(mm, nn2, g))
            # m12/13 run inside segment B with the split-K (no-hop) path;
            # y writes on the gpsimd queue to keep sync clear
            for j in range(4):
                m, nn = 12 + j // 2, j % 2
                place(660, 136, 143,
                      lambda g, mm=m, nn2=nn: emit_proj_split(
                          mm, nn2, g, stg7, (mm - 12) * P, y_q=nc.gpsimd))

            # ---------------- prefix + boot ----------------
            # kq c0 m0 interleaved per k-slice (consumes xt chunk-0 as the
            # two DMA halves land), then v for chunks 0-1 fills the PE
            # during the remaining DMA window.
            kst, qst = {}, {}
            pin_mm = None
            for k in range(KD):
                mm = emit_kq_piece("k", wk_sb, kt_sb, 0, 0, [k], kst)
                if pin_mm is None:
                    pin_mm = mm
                emit_kq_piece("q", wq_sb, qt_sb, 0, 0, [k], qst)

            # wave-2: the rest, gated on the first prefix matmul (= wave-1
            # landed) so it doesn't delay the critical set; issue order
            # matches block-0's consumption order (xt c1, c2, c3, weights)
            # wave-2 as many small pieces in consumption order: with several
            # transfers in flight per queue the engines overlap them and
            # aggregate bandwidth rises well above the ~105GB/s per-transfer
            # serial rate, pulling xt c1-3 arrival earlier for block 0
            wave2 = []
            for c in (1, 2, 3):
                for k2 in range(0, KD, 2):
                    eng = nc.sync if (k2 // 2) % 2 == 0 else nc.scalar
                    wave2.append((eng, xt_sb[:, c, k2:k2 + 2],
                                  xt[:, c, k2:k2 + 2]))
            wave2 += [(nc.sync, wt_sb[:, :], wt[:, :]),
                      (nc.scalar, wo_sb[:, 0], wo[:, 0]),
                      (nc.sync, wo_sb[:, 1], wo[:, 1])]
            for dma_eng, dst, src in wave2:
                dma = dma_eng.dma_start(dst, src)
                bass._add_dep_helper(dma.ins, pin_mm.ins, sync=True,
                                     reason="defer wave2 dma")

            sts = {}
            sts[0] = emit_scores(0, 0)
            # v tile 0 in the boot, order-pinned behind the first scores so
            # the scheduler can't push the scores behind it
            emit_v_piece(0, list(range(KD)), after=last_scores["mm"])

            # ---------------- main pipeline ----------------
            o_ps = None
            last_pv = None
            pend_norm = None          # (seg, o_sb, rc) of the previous segment
            for u in range(NSTEP):
                seg, t = u // 16, u % 16
                if t == 0:
                    o_ps = alloc_o(seg)
                if u + 1 < NSTEP:
                    sts[u + 1] = emit_scores((u + 1) // 16, (u + 1) % 16)
                e = emit_exp(seg, t, sts.pop(u))
                for fn in extras[u]:
                    fn(last_pv)
                # previous segment's normalization at t=4/6: the reciprocal
                # roundtrip issued at its t=15 has ~4 steps to land, and the
                # early norm opens the proj window for the previous chunk
                if pend_norm is not None and t in (4, 6):
                    pseg, po_sb, prc = pend_norm
                    if pseg == NSEG - 2:
                        emit_norm_nohop(pseg, po_sb, prc, 0 if t == 4 else 1)
                    else:
                        emit_epi_norm(pseg, po_sb, prc, 0 if t == 4 else 1, 2)
                    if t == 6:
                        pend_norm = None
                last_pv = emit_pv(seg, t, e, o_ps)
                if t == NT - 1 and u + 1 < NSTEP:
                    o_sb, rc = emit_epi_stage(seg, o_ps)
                    pend_norm = (seg, o_sb, rc)

            # ---------------- tail: segment B (= seg 8) epilogue ----------
            # after the last exp only ~512 denominators + proj m14/15 remain.
            # No DMA roundtrips: ScalarE (idle now; one table switch) takes
            # the reciprocal straight off the PSUM denominator rows, ONE
            # K=1 matmul broadcasts [1,512] -> [64,512] PSUM, and the ks=1
            # contraction is split K=64+K=64 against a pre-shifted wo copy
            # so the half-1 numerators never need a partition-shift DMA.
            cT, prT, q0B, q1B = SEGS[NSEG - 1]
            wB = q1B - q0B
            # 1/d = exp(-ln d): Ln and Exp live in different table sets, so
            # the order MUST be Ln,Ln,Exp,Exp (pinned) — walrus inserts a
            # ~1.3us ACT_TABLE_LOAD at every Ln<->Exp alternation otherwise.
            # The activations are emitted BEFORE the o_sb staging copies so
            # nothing sits ahead of them in any queue.
            lnv = epi.tile([P, 2, wB], F32, tag="lnv", name="lnv_t")
            rinv = epi.tile([P, 2, wB], BF16, tag="rinv", name="rinv_t")
            prev_act = None
            for half in range(2):
                a = nc.scalar.activation(
                    lnv[HD:HD + 1, half, :], o_ps[half][HD:HD + 1, 0:wB],
                    mybir.ActivationFunctionType.Ln,
                )
                if prev_act is not None:
                    bass._add_dep_helper(a.ins, prev_act.ins, sync=False,
                                         reason="table set order")
                prev_act = a
            for half in range(2):
                a = nc.scalar.activation(
                    rinv[HD:HD + 1, half, :], lnv[HD:HD + 1, half, :],
                    Exp, scale=-1.0,
                )
                bass._add_dep_helper(a.ins, prev_act.ins, sync=False,
                                     reason="table set order")
                prev_act = a
            o_sb = epi.tile([HD + 1, 2, wB], F32, tag=f"osb{wB}",
                            name="osb_tail")
            for half in range(2):
                nc.vector.tensor_copy(o_sb[:, half, :], o_ps[half][:, 0:wB])
            rbs_ps = ps_o.tile([P, QC], F32, tag="o", name="rbs_ps")
            nc.tensor.matmul(
                rbs_ps[0:HD, 0:2 * wB], ones_t[HD:HD + 1, :],
                rinv[HD:HD + 1, :, :], start=True, stop=True,
            )
            # proj ks0 + keepwarms fill the PE while the reciprocal runs
            tail_ps = [ps_st.tile([P, 2, QC], F32, tag="st", name=f"tps{i}")
                       for i in range(2)]
            slots = [tail_ps[0][:, 0, :], tail_ps[0][:, 1, :],
                     tail_ps[1][:, 0, :], tail_ps[1][:, 1, :]]
            for j in range(4):
                m, nn = 14 + j // 2, j % 2
                mm = nc.tensor.matmul(
                    slots[j], at_sb[:, 0, m * P:(m + 1) * P],
                    wo_sb[:, 0, nn * QC:(nn + 1) * QC], start=True, stop=False,
                )
                bass._add_dep_helper(mm.ins, last_pv.ins, sync=False,
                                     reason="tail ks0")
            warm3 = ps_a.tile([P, QC], F32, tag="a", name="warm3")
            for _ in range(55):
                mm = nc.tensor.matmul(
                    warm3[:, 0:P], warm_w, warm_w, start=True, stop=True
                )
                bass._add_dep_helper(mm.ins, last_pv.ins, sync=False,
                                     reason="tail keepwarm")
            # piece-pipelined: normalize 128 at-columns then immediately the
            # proj unit that consumes them; casts/y spread across engines
            stg1 = epi.tile([HD, wB], BF16, tag="stg1", name="stg1_t")
            for p in range(2):
                wp = P
                ss = slice(p * wp, (p + 1) * wp)
                a0 = cT * QC + q0B + p * wp
                m = 14 + p
                nc.vector.tensor_mul(
                    stg1[:, ss], o_sb[0:HD, 1, ss],
                    rbs_ps[0:HD, wB + p * wp:wB + (p + 1) * wp],
                )
                nc.vector.tensor_mul(
                    at_sb[0:HD, prT, a0:a0 + wp], o_sb[0:HD, 0, ss],
                    rbs_ps[0:HD, p * wp:(p + 1) * wp],
                )
                for nn in range(2):
                    j = 2 * p + nn
                    nc.tensor.matmul(
                        slots[j], at_sb[0:HD, 1, m * P:(m + 1) * P],
                        wo_sb[0:HD, 1, nn * QC:(nn + 1) * QC],
                        start=False, stop=False,
                    )
                    nc.tensor.matmul(
                        slots[j], stg1[:, ss],
                        wt_sb[:, nn * QC:(nn + 1) * QC],
                        start=False, stop=True,
                    )
                    ysb = outp.tile([P, QC], BF16, tag="y", name=f"yt_{m}_{nn}")
                    if j % 2 == 0:
                        nc.vector.tensor_copy(ysb[:], slots[j])
                    else:
                        nc.scalar.copy(ysb[:], slots[j])
                    yq = (nc.sync, nc.gpsimd, nc.gpsimd, nc.sync)[j]
                    yq.dma_start(y_r[:, m, nn * QC:(nn + 1) * QC], ysb[:])

    nc.finalize()
    return nc


_NC = None


def _get_nc():
    global _NC
    if _NC is None:
        _NC = build_nc()
    return _NC


def _swiz_w(w):
    # [1024, cols] -> [128, 8, cols]: partition-contiguous for 1-desc rows
    return np.ascontiguousarray(
        w.reshape(KD, P, w.shape[1]).transpose(1, 0, 2)
    )


def _in_maps(x, w_qkv, w_out):
    bf = ml_dtypes.bfloat16
    x = np.asarray(x, dtype=np.float32)
    w_qkv = np.asarray(w_qkv, dtype=np.float32)
    w_out = np.asarray(w_out, dtype=np.float32)
    # xt[p, chunk, k, n'] = x[b].T[k*128+p, chunk*512+n']
    xts = []
    for b in range(2):
        xtb = x[b].T.reshape(KD, P, NQC, QC).transpose(1, 2, 0, 3)
        xts.append(np.ascontiguousarray(xtb).astype(bf))
    wq_g = [_swiz_w(w_qkv[:, 0 * D + g * DG:0 * D + (g + 1) * DG]).astype(bf) for g in range(4)]
    wk_g = [_swiz_w(w_qkv[:, 1 * D + g * DG:1 * D + (g + 1) * DG]).astype(bf) for g in range(4)]
    wv_g = [_swiz_w(w_qkv[:, 2 * D + g * DG:2 * D + (g + 1) * DG]).astype(bf) for g in range(4)]
    wo_g = [
        np.ascontiguousarray(
            w_out[g * DG:(g + 1) * DG, :].reshape(2, P, D).transpose(1, 0, 2)
        ).astype(bf)
        for g in range(4)
    ]
    wt_g = [np.ascontiguousarray(wo_g[g][64:128, 1, :]) for g in range(4)]
    maps = []
    for c in range(8):
        b, g = c // 4, c % 4
        maps.append({
            "xt": xts[b],
            "wq": wq_g[g],
            "wk": wk_g[g],
            "wv": wv_g[g],
            "wo": wo_g[g],
            "wt": wt_g[g],
        })
    return maps


LAST_RESULT = None


def kernel(x, w_qkv, w_out, b_out):
    from concourse.bass_utils import run_bass_kernel_spmd

    nc = _get_nc()
    maps = _in_maps(x, w_qkv, w_out)
    res = run_bass_kernel_spmd(nc, maps, list(range(8)))
    global LAST_RESULT
    LAST_RESULT = res
    out = np.zeros((2, N, D), dtype=np.float32)
    for c in range(8):
        out[c // 4] += np.asarray(res.results[c]["y"], dtype=np.float32)
    out += np.asarray(b_out, dtype=np.float32)[None, None, :]
    return out


# revision 65
# speedup vs baseline: 1.0156x; 1.0156x over previous
"""Multi-head attention (b=2, n=2048, d=1024, H=16 heads) on 8 TRN2 NeuronCores.

Sharding: core c = (b, g) with b = c // 4 (data parallel over batch) and
g = c % 4 (tensor parallel over head groups of 4 heads).  Each core computes
qkv projections for its 4 heads, full softmax attention for those heads, and
a partial output projection y_partial = A_heads @ w_out[g*256:(g+1)*256].
The host sums the 4 partials per batch and adds b_out.

v2 schedule (from trace analysis of the 214us baseline):
  - ScalarE exp is the binding engine (128 x 1114ns ACTIVATE); runtime =
    boot + gap-free exp stream + tail, so the whole design minimizes the
    three non-exp components.
  - boot: input DMA was ISSUE-bound (34 dma_starts x ~650ns serialized on
    the sync queue).  Now ~9 coalesced issues split across the sync AND
    scalar queues (both are HWDGE engines; transfers stripe over all 16
    DMA engines regardless).  v-projection for chunks 0-1 and the kq
    prefix fill the PE during the DMA window.
  - pr-major block order (0,0),(1,0),(2,0),(3,0),(0,1).. so kt m1 is not
    needed until step 64 (c-major needed ALL of kt by step 16, overloading
    block 0 by ~6us).  JIT kq groups spread at 2 matmuls/step via a greedy
    per-step load balancer (4-matmul bursts overflowed the ~360ns/step PE
    slack and stalled exp by ~600ns once per block).
  - norm of block B runs during B+1 at t=4/6 (not 12/14): with pr-major
    there is no DVE congestion early in a block, and the earlier norm
    opens a 10-step window for the out-proj units of the previous chunk.
  - out-proj units are atomic (ks0+ks1 back to back) gated on both halves
    of their at-chunk; pinned behind the attention stream as before.
  - tail: the last block (3,1) is split into two 256-col segments A/B.
    A's epilogue (denominator DMA-transpose roundtrip, norm, proj m12/13,
    casts, y writes) hides behind B's 16-step exp stream; only B's own
    epilogue (~512 denominators) plus proj m14/15 remain exposed after the
    last exp.  Fast path for the tail denominators: ScalarE (idle after
    the last exp) copies the PSUM denominator rows out immediately, the
    gather/scatter DMAs issue from scalar+sync+gpsimd queues in parallel,
    and const matmuls keep the PE p-state up through the reciprocal
    roundtrip.
Matmuls run in bf16 (fp32 PSUM accumulation), y written bf16.
"""

import os
import sys

for _p in ("/opt/trn_rl_repo",):
    if _p not in sys.path and os.path.isdir(_p):
        sys.path.insert(0, _p)

import ml_dtypes
import numpy as np

import concourse.bass as bass
import concourse.mybir as mybir
import concourse.tile as tile
from concourse import bacc

P = 128
D = 1024          # model dim
N = 2048          # sequence length
HD = 64           # head dim
GH = 4            # heads per core
DG = GH * HD      # 256 projected cols per core
KD = D // P       # 8 k-tiles over model dim
NT = N // P       # 16 tiles over sequence
QC = 512          # n_q chunk size
NQC = N // QC     # 4 chunks
SCALE = HD ** -0.5

F32 = mybir.dt.float32
BF16 = mybir.dt.bfloat16

Exp = mybir.ActivationFunctionType.Exp

# segments: (chunk, pr, q0, q1) — 7 full blocks pr-major, last block split
# into two 256-col halves so its epilogue pipelines with compute.
SEGS = [(0, 0, 0, QC), (1, 0, 0, QC), (2, 0, 0, QC), (3, 0, 0, QC),
        (0, 1, 0, QC), (1, 1, 0, QC), (2, 1, 0, QC),
        (3, 1, 0, 256), (3, 1, 256, QC)]
NSEG = len(SEGS)
NSTEP = 16 * NSEG  # 144


def build_nc():
    nc = bacc.Bacc("TRN2")

    # all inputs pre-swizzled on host to [128 partitions, ...contiguous]
    xt = nc.declare_dram_parameter("xt", [P, NQC, KD, QC], BF16, isOutput=False)
    wq = nc.declare_dram_parameter("wq", [P, KD, DG], BF16, isOutput=False)
    wk = nc.declare_dram_parameter("wk", [P, KD, DG], BF16, isOutput=False)
    wv = nc.declare_dram_parameter("wv", [P, KD, DG], BF16, isOutput=False)
    wo = nc.declare_dram_parameter("wo", [P, 2, D], BF16, isOutput=False)
    # wo rows 192-255 (ks=1 upper half) pre-shifted to partitions 0-63: the
    # tail projection contracts them against un-hopped half-1 numerators
    wt = nc.declare_dram_parameter("wt", [HD, D], BF16, isOutput=False)
    y = nc.declare_dram_parameter("y", [N, D], BF16, isOutput=True)

    y_r = y[:, :].rearrange("(o p) n -> p o n", p=P)      # [128, 16, 1024]

    with tile.TileContext(nc) as tc, nc.allow_low_precision("bf16 attention"):
        with (
            tc.tile_pool(name="wpool", bufs=1) as wpool,
            tc.tile_pool(name="qkvpool", bufs=1) as qkvpool,
            tc.tile_pool(name="attnpool", bufs=1) as attnpool,
            tc.tile_pool(name="xpool", bufs=1) as xpool,
            tc.tile_pool(name="work", bufs=6) as work,
            tc.tile_pool(name="epi", bufs=2) as epi,
            tc.tile_pool(name="outp", bufs=8) as outp,
            tc.tile_pool(name="ps_a", bufs=2, space="PSUM") as ps_a,
            tc.tile_pool(name="ps_st", bufs=2, space="PSUM") as ps_st,
            tc.tile_pool(name="ps_o", bufs=2, space="PSUM") as ps_o,
        ):
            # --- persistent SBUF tiles ---
            wk_sb = wpool.tile([P, KD, DG], BF16, tag="wk")
            wq_sb = wpool.tile([P, KD, DG], BF16, tag="wq")
            wv_sb = wpool.tile([P, KD, DG], BF16, tag="wv")
            wo_sb = wpool.tile([P, 2, D], BF16, tag="wo")
            xt_sb = xpool.tile([P, NQC, KD, QC], BF16, tag="xt")

            wt_sb = wpool.tile([HD, D], BF16, tag="wt")
            qt_sb = qkvpool.tile([P, 2, N], BF16, tag="qt")   # [256, 2048] qT
            kt_sb = qkvpool.tile([P, 2, N], BF16, tag="kt")   # [256, 2048] kT
            vg_sb = qkvpool.tile([P, NT, GH, 66], BF16, tag="vg")  # v + ones
            at_sb = attnpool.tile([P, 2, N], BF16, tag="at")  # attn_outT

            # --- coalesced input DMA, issues split over sync+scalar queues
            # (each dma_start costs ~650ns of issue time on its queue and
            # its transfer stripes across all 16 DMA engines; the baseline's
            # 34 issues serialized ~20us on the sync queue alone) ---
            # --- wave-1 DMA: just the critical set for the first scores
            # (transfers on one queue run near-serially at ~105GB/s each;
            # two HWDGE queues give ~2x.  Anything queued alongside the
            # critical set delays it 1:1, so the rest waits in wave 2.)
            nc.sync.dma_start(xt_sb[:, 0, 0:4], xt[:, 0, 0:4])
            nc.scalar.dma_start(xt_sb[:, 0, 4:8], xt[:, 0, 4:8])
            nc.sync.dma_start(wk_sb[:, 0:4], wk[:, 0:4])
            nc.scalar.dma_start(wq_sb[:, 0:4], wq[:, 0:4])
            nc.sync.dma_start(wk_sb[:, 4:8], wk[:, 4:8])
            nc.scalar.dma_start(wq_sb[:, 4:8], wq[:, 4:8])
            nc.sync.dma_start(wv_sb[:, 0:4], wv[:, 0:4])
            nc.scalar.dma_start(wv_sb[:, 4:8], wv[:, 4:8])

            nc.scalar.copy(
                vg_sb[:, :, :, HD:], nc.const_aps.tensor(1.0, (P, NT, GH, 2), F32)
            )
            ones_t = wpool.tile([P, HD], BF16, tag="ones_t")
            nc.scalar.copy(
                ones_t[HD:HD + 1, :], nc.const_aps.tensor(1.0, (1, HD), F32)
            )

            # PE p-state warmup while the wave-1 DMAs land (~80 x ~80ns
            # covers the window to data arrival without overrunning it)
            warm_w = nc.const_aps.tensor(1.0, (P, P), BF16)
            warm_ps = ps_st.tile([P, 2, QC], F32, tag="st", name="warm")
            for _ in range(120):
                nc.tensor.matmul(
                    warm_ps[:, 0, 0:P], warm_w, warm_w, start=True, stop=True
                )

            # ---------------- emitters ----------------
            def emit_kq_piece(which, w_sb, dst, m, c, ks, state):
                # resumable slice of an 8-matmul k/q projection group
                if state.get("ps") is None:
                    state["ps"] = ps_a.tile(
                        [P, QC], F32, tag="a", name=f"{which}ps_{m}_{c}"
                    )
                ps = state["ps"]
                mm = None
                for k in ks:
                    mm = nc.tensor.matmul(
                        ps[:],
                        w_sb[:, k, m * P:(m + 1) * P],
                        xt_sb[:, c, k, :],
                        start=(k == 0),
                        stop=(k == KD - 1),
                    )
                if ks[-1] == KD - 1:
                    nc.vector.tensor_copy(dst[:, m, c * QC:(c + 1) * QC], ps[:])
                return mm

            def emit_v_piece(t, ks, after=None):
                c4, r4 = t // 4, t % 4
                ps = v_state.get(t)
                if ps is None:
                    ps = ps_a.tile([P, QC], F32, tag="a", name=f"vps_{t}")
                    v_state[t] = ps
                mm = None
                for k in ks:
                    mm = nc.tensor.matmul(
                        ps[:, :DG],
                        xt_sb[:, c4, k, r4 * P:(r4 + 1) * P],
                        wv_sb[:, k, :],
                        start=(k == 0),
                        stop=(k == KD - 1),
                    )
                    if after is not None:
                        bass._add_dep_helper(mm.ins, after.ins, sync=False,
                                             reason="order after scores")
                        after = None
                if ks[-1] == KD - 1:
                    nc.vector.tensor_copy(
                        vg_sb[:, t, :, 0:HD],
                        ps[:, :DG].rearrange("p (h e) -> p h e", h=GH),
                    )
                return mm

            v_state = {}

            last_scores = {}

            def emit_scores(seg, t):
                c, pr, q0, q1 = SEGS[seg]
                cs = slice(c * QC + q0, c * QC + q1)
                ts_ = slice(t * P, (t + 1) * P)
                st = ps_st.tile([P, 2, QC], F32, tag="st", name=f"st_{seg}_{t}")
                for half in range(2):
                    hs = slice(half * HD, (half + 1) * HD)
                    last_scores["mm"] = nc.tensor.matmul(
                        st[:, half, 0:q1 - q0],
                        kt_sb[hs, pr, ts_],
                        qt_sb[hs, pr, cs],
                        start=True,
                        stop=True,
                    )
                return st

            def emit_exp(seg, t, st):
                c, pr, q0, q1 = SEGS[seg]
                w = q1 - q0
                e = work.tile([P, 2, w], BF16, tag=f"exp{w}", name=f"e_{seg}_{t}")
                nc.scalar.activation(e[:], st[:, :, 0:w], Exp, scale=SCALE)
                return e

            def emit_pv(seg, t, e, o_ps):
                c, pr, q0, q1 = SEGS[seg]
                last = None
                for half in range(2):
                    h = 2 * pr + half
                    last = nc.tensor.matmul(
                        o_ps[half][:, 0:q1 - q0],
                        vg_sb[:, t, h, 0:HD + 1],
                        e[:, half, :],
                        start=(t == 0),
                        stop=(t == NT - 1),
                    )
                return last

            def alloc_o(seg):
                o_ps = []
                for half in range(2):
                    o_full = ps_o.tile(
                        [P, QC], F32, tag="o", name=f"o_{seg}_{half}"
                    )
                    o_ps.append(o_full[: HD + 1])
                return o_ps

            def emit_epi_stage(seg, o_ps, tail=False):
                # stage PV accumulators to SBUF; invert the denominators
                # TRANSPOSED via a DMA gather to [128, w//64] (1024 denoms on
                # one partition would cost ~7ns/elem on the DVE), reciprocal,
                # scatter back to partition 0 for the gpsimd broadcast.
                c, pr, q0, q1 = SEGS[seg]
                w = q1 - q0
                i8 = w // 64
                o_sb = epi.tile([HD + 1, 2, w], F32, tag=f"osb{w}",
                                name=f"osb_{seg}")
                dT = epi.tile([P, i8], F32, tag=f"dT{i8}", name=f"dT_{seg}")
                if tail:
                    # ScalarE is idle after the last exp: pull the denom rows
                    # out fast so the gather doesn't wait for the full o_sb
                    # copies.  Wide-row gather/scatter shapes ([w//32, 32]
                    # instead of [*, w//64]) cut the DMA descriptor count
                    # ~8x, and the three DMAs issue on three queues.
                    rows = w // 32
                    den = epi.tile([HD + 1, 2, w], F32, tag=f"den{w}",
                                   name=f"den_{seg}")
                    dTt = epi.tile([2 * rows, 32], F32, tag="dTt",
                                   name=f"dTt_{seg}")
                    for half in range(2):
                        nc.scalar.copy(
                            den[HD:HD + 1, half, :], o_ps[half][HD:HD + 1, 0:w]
                        )
                    nc.scalar.dma_start(
                        dTt[0:rows, :],
                        den[HD:HD + 1, 0, :].rearrange("o (b i) -> o b i", i=32),
                    )
                    nc.sync.dma_start(
                        dTt[rows:2 * rows, :],
                        den[HD:HD + 1, 1, :].rearrange("o (b i) -> o b i", i=32),
                    )
                    rcTt = epi.tile([2 * rows, 32], F32, tag="rcTt",
                                    name=f"rcTt_{seg}")
                    nc.vector.reciprocal(rcTt[:], dTt[:, :])
                    rc = epi.tile([1, 2, w], F32, tag=f"rc{w}",
                                  name=f"rct_{seg}")
                    nc.gpsimd.dma_start(
                        rc[0:1, :, :].rearrange("o h (b i) -> o (h b) i", i=32),
                        rcTt[:, :],
                    )
                    for half in range(2):
                        nc.vector.tensor_copy(o_sb[:, half, :],
                                              o_ps[half][:, 0:w])
                    return o_sb, rc
                else:
                    for half in range(2):
                        nc.vector.tensor_copy(o_sb[:, half, :],
                                              o_ps[half][:, 0:w])
                        nc.sync.dma_start(
                            dT[HD * half:HD * (half + 1), :],
                            o_sb[HD:HD + 1, half, :].rearrange(
                                "o (b i) -> o b i", i=i8
                            ),
                        )
                rcT = epi.tile([P, i8], F32, tag=f"rcT{i8}", name=f"rcT_{seg}")
                nc.vector.reciprocal(rcT[:], dT[:, :])
                rc = epi.tile([1, 2, w], F32, tag=f"rc{w}", name=f"rc_{seg}")
                nc.sync.dma_start(
                    rc[0:1, :, :].rearrange("o h (b i) -> o (h b) i", i=i8),
                    rcT[:, :],
                )
                return o_sb, rc

            def emit_epi_norm(seg, o_sb, rc, s, pieces):
                # one piece of the normalization: A^T = o[:64]*(1/o[64]).
                c, pr, q0, q1 = SEGS[seg]
                wseg = q1 - q0
                rbs = epi.tile([HD, 2, wseg], F32, tag=f"rbs{wseg}",
                               name=f"rbs_{seg}", uniquify=True)
                w = wseg // pieces
                ss = slice(s * w, (s + 1) * w)
                a0 = c * QC + q0 + s * w
                nc.gpsimd.partition_broadcast(
                    rbs[:, :, ss], rc[0:1, :, ss], channels=HD
                )
                # half 1 first: its at-write goes through an extra SBUF->SBUF
                # DMA hop (partition shift), so start that chain earliest
                stg = work.tile(
                    [HD, w], BF16, tag=f"stg{w}", name=f"stg_{seg}_{s}"
                )
                nc.vector.tensor_mul(stg[:], o_sb[0:HD, 1, ss], rbs[:, 1, ss])
                nc.sync.dma_start(at_sb[HD:P, pr, a0:a0 + w], stg[:])
                nc.vector.tensor_mul(
                    at_sb[0:HD, pr, a0:a0 + w],
                    o_sb[0:HD, 0, ss],
                    rbs[:, 0, ss],
                )

            # seg 7 (tail-A): half-1 numerators stay on partitions 0-63 in
            # stg7 (no SBUF->SBUF partition-shift DMA: those land on a slow
            # table queue whose drain gates kernel completion), and the
            # m12/13 proj contracts them against the pre-shifted wt copy
            stg7 = epi.tile([HD, 256], BF16, tag="stg7", name="stg7")

            def emit_norm_nohop(seg, o_sb, rc, s):
                c, pr, q0, q1 = SEGS[seg]
                w = (q1 - q0) // 2
                rbs = epi.tile([HD, 2, q1 - q0], F32, tag="rbsA",
                               name=f"rbsA_{seg}", uniquify=True)
                ss = slice(s * w, (s + 1) * w)
                a0 = c * QC + q0 + s * w
                nc.gpsimd.partition_broadcast(
                    rbs[:, :, ss], rc[0:1, :, ss], channels=HD
                )
                nc.vector.tensor_mul(stg7[:, ss], o_sb[0:HD, 1, ss],
                                     rbs[:, 1, ss])
                nc.vector.tensor_mul(
                    at_sb[0:HD, pr, a0:a0 + w], o_sb[0:HD, 0, ss],
                    rbs[:, 0, ss],
                )

            def emit_proj_split(m, nn, gate, stg, soff, y_q=None):
                # proj unit with the ks=1 contraction split K=64+K=64 so the
                # half-1 numerators are read from stg (partitions 0-63)
                ps = ps_a.tile([P, QC], F32, tag="a", name=f"yps_{m}_{nn}")
                mm = nc.tensor.matmul(
                    ps[:], at_sb[:, 0, m * P:(m + 1) * P],
                    wo_sb[:, 0, nn * QC:(nn + 1) * QC], start=True, stop=False,
                )
                if gate is not None:
                    bass._add_dep_helper(mm.ins, gate.ins, sync=False,
                                         reason="defer proj")
                nc.tensor.matmul(
                    ps[:], at_sb[0:HD, 1, m * P:(m + 1) * P],
                    wo_sb[0:HD, 1, nn * QC:(nn + 1) * QC],
                    start=False, stop=False,
                )
                nc.tensor.matmul(
                    ps[:], stg[:, soff:soff + P],
                    wt_sb[:, nn * QC:(nn + 1) * QC], start=False, stop=True,
                )
                ysb = outp.tile([P, QC], BF16, tag="y", name=f"y_{m}_{nn}")
                nc.vector.tensor_copy(ysb[:], ps[:])
                (y_q or nc.sync).dma_start(
                    y_r[:, m, nn * QC:(nn + 1) * QC], ysb[:]
                )

            def emit_proj_unit(m, nn, gate, cast_eng=None, y_q=None):
                # atomic out-proj unit: both ks matmuls, drain cast, y write.
                # order-pinned behind the attention stream (the sim
                # undercosts the DVE reciprocal chain feeding at_sb, and an
                # optimistically-early proj in the static PE queue
                # head-of-line blocks the scores on hardware)
                ps = ps_a.tile([P, QC], F32, tag="a", name=f"yps_{m}_{nn}")
                for ks in range(2):
                    mm = nc.tensor.matmul(
                        ps[:],
                        at_sb[:, ks, m * P:(m + 1) * P],
                        wo_sb[:, ks, nn * QC:(nn + 1) * QC],
                        start=(ks == 0),
                        stop=(ks == 1),
                    )
                    if gate is not None:
                        bass._add_dep_helper(mm.ins, gate.ins, sync=False,
                                             reason="defer proj")
                ysb = outp.tile([P, QC], BF16, tag="y", name=f"y_{m}_{nn}")
                if cast_eng == "scalar":
                    nc.scalar.copy(ysb[:], ps[:])
                else:
                    nc.vector.tensor_copy(ysb[:], ps[:])
                (y_q or nc.sync).dma_start(
                    y_r[:, m, nn * QC:(nn + 1) * QC], ysb[:]
                )

            # ---------------- extras load balancer ----------------
            # load[u] = (base PE cost) - (ACT budget): minimizing load
            # equalizes *overflow*, so extras prefer steps with headroom
            # (the 256-wide tail segments have short exps = little slack).
            load = [0.0] * NSTEP
            for u in range(NSTEP):
                _, _, q0, q1 = SEGS[u // 16]
                w = q1 - q0
                load[u] = (220.0 + 2 * w * 0.42 + 80.0) - (172 + 2 * w) / 1.2
            extras = {u: [] for u in range(NSTEP)}

            def place(cost, est, lst, fn):
                best = min(range(est, lst + 1), key=lambda u: load[u])
                load[best] += cost
                extras[best].append(fn)
                return best

            def sched_group(cost, est, lst, fns):
                # pieces of one group on CONSECUTIVE steps (the group holds
                # a ps_a buf from first piece to last; adjacency bounds the
                # hold time so the 2-buf pool doesn't starve)
                n = len(fns)
                b0 = min(range(est, lst - n + 2),
                         key=lambda u: max(load[u + i] for i in range(n)))
                for i, fn in enumerate(fns):
                    load[b0 + i] += cost
                    extras[b0 + i].append(fn)

            # v tiles 1-15 inside block 0's steps so they interleave with the
            # scores/PV stream instead of sitting as one blob in the PE FIFO
            # (tile 0 runs in the boot, pinned behind the first scores).
            # est follows each chunk's wave-2 DMA arrival — work placed in
            # the FIFO before its data exists head-of-line blocks the PE.
            V_EST = {0: 0, 1: 2, 2: 4, 3: 6}   # per xt chunk
            for t in (1, 2):
                place(880, t - 1, t - 1,
                      lambda gate, tt=t: emit_v_piece(tt, list(range(KD))))
            for t in range(3, NT):
                fns = [
                    lambda gate, tt=t: emit_v_piece(tt, [0, 1, 2, 3]),
                    lambda gate, tt=t: emit_v_piece(tt, [4, 5, 6, 7]),
                ]
                sched_group(440, min(V_EST[t // 4], t - 2), t - 1, fns)
            # kt m0 c1-3 feed block 0's own scores at t=4/8/12: emitted
            # before those scores but not before their xt chunk arrives
            for c, est, lst in ((1, 1, 2), (2, 4, 6), (3, 6, 10)):
                st_c = {}
                sched_group(880, est, lst, [
                    lambda gate, cc=c, st=st_c: emit_kq_piece(
                        "k", wk_sb, kt_sb, 0, cc, [0, 1, 2, 3], st),
                    lambda gate, cc=c, st=st_c: emit_kq_piece(
                        "k", wk_sb, kt_sb, 0, cc, [4, 5, 6, 7], st),
                ])

            # JIT kq groups at 2-matmul granularity.  kq group (which, m, c)
            # produces keys/queries consumed from step `need` onward.
            def kq_fns(which, w_sb, dst, m, c):
                state = {}
                return [
                    lambda gate, st=state, kk=k: emit_kq_piece(
                        which, w_sb, dst, m, c, [kk, kk + 1], st
                    )
                    for k in (0, 2, 4, 6)
                ]

            sched_group(440, 3, 14, kq_fns("q", wq_sb, qt_sb, 0, 1))
            sched_group(440, 16, 30, kq_fns("q", wq_sb, qt_sb, 0, 2))
            sched_group(440, 32, 46, kq_fns("q", wq_sb, qt_sb, 0, 3))
            sched_group(440, 48, 62, kq_fns("q", wq_sb, qt_sb, 1, 0))
            sched_group(440, 64, 78, kq_fns("q", wq_sb, qt_sb, 1, 1))
            sched_group(440, 80, 94, kq_fns("q", wq_sb, qt_sb, 1, 2))
            sched_group(440, 96, 110, kq_fns("q", wq_sb, qt_sb, 1, 3))
            # kt m1: needed from step 64 (seg 4 = first pr=1 block); c3 keys
            # (1536+) first touched at seg4 t12 = step 76
            sched_group(440, 16, 40, kq_fns("k", wk_sb, kt_sb, 1, 0))
            sched_group(440, 24, 52, kq_fns("k", wk_sb, kt_sb, 1, 1))
            sched_group(440, 40, 60, kq_fns("k", wk_sb, kt_sb, 1, 2))
            sched_group(440, 48, 70, kq_fns("k", wk_sb, kt_sb, 1, 3))

            # out-proj: chunk c's at is complete once norm of its pr=1 block
            # (seg 4+c) lands during seg 5+c at t=4/6 -> window opens t8.
            # chunk 3: m12/13 open after seg7(A)'s norm early in seg8;
            # m14/15 run in the tail.
            proj_windows = {0: (88, 119), 1: (104, 131), 2: (120, 139)}
            for c in range(3):
                est, lst = proj_windows[c]
                for j in range(8):
                    m, nn = 4 * c + j // 2, j % 2
                    place(440, est, lst,
                          lambda g, mm=m, nn2=nn: emit_proj_unit(mm, nn2, g))
            # m12/13 run inside segment B with the split-K (no-hop) path;
            # y writes on the gpsimd queue to keep sync clear
            for j in range(4):
                m, nn = 12 + j // 2, j % 2
                place(660, 136, 143,
                      lambda g, mm=m, nn2=nn: emit_proj_split(
                          mm, nn2, g, stg7, (mm - 12) * P, y_q=nc.gpsimd))

            # ---------------- prefix + boot ----------------
            # kq c0 m0 interleaved per k-slice (consumes xt chunk-0 as the
            # two DMA halves land), then v for chunks 0-1 fills the PE
            # during the remaining DMA window.
            kst, qst = {}, {}
            pin_mm = None
            for k in range(KD):
                mm = emit_kq_piece("k", wk_sb, kt_sb, 0, 0, [k], kst)
                if pin_mm is None:
                    pin_mm = mm
                emit_kq_piece("q", wq_sb, qt_sb, 0, 0, [k], qst)

            # wave-2: the rest, gated on the first prefix matmul (= wave-1
            # landed) so it doesn't delay the critical set; issue order
            # matches block-0's consumption order (xt c1, c2, c3, weights)
            for dma_eng, dst, src in (
                (nc.sync, xt_sb[:, 1, 0:4], xt[:, 1, 0:4]),
                (nc.scalar, xt_sb[:, 1, 4:8], xt[:, 1, 4:8]),
                (nc.sync, xt_sb[:, 2, 0:4], xt[:, 2, 0:4]),
                (nc.scalar, xt_sb[:, 2, 4:8], xt[:, 2, 4:8]),
                (nc.sync, xt_sb[:, 3, 0:4], xt[:, 3, 0:4]),
                (nc.scalar, xt_sb[:, 3, 4:8], xt[:, 3, 4:8]),
                (nc.sync, wt_sb[:, :], wt[:, :]),
                (nc.scalar, wo_sb[:, :, :], wo[:, :, :]),
            ):
                dma = dma_eng.dma_start(dst, src)
                bass._add_dep_helper(dma.ins, pin_mm.ins, sync=True,
                                     reason="defer wave2 dma")

            sts = {}
            sts[0] = emit_scores(0, 0)
            # v tile 0 in the boot, order-pinned behind the first scores so
            # the scheduler can't push the scores behind it
            emit_v_piece(0, list(range(KD)), after=last_scores["mm"])

            # ---------------- main pipeline ----------------
            o_ps = None
            last_pv = None
            pend_norm = None          # (seg, o_sb, rc) of the previous segment
            for u in range(NSTEP):
                seg, t = u // 16, u % 16
                if t == 0:
                    o_ps = alloc_o(seg)
                if u + 1 < NSTEP:
                    sts[u + 1] = emit_scores((u + 1) // 16, (u + 1) % 16)
                e = emit_exp(seg, t, sts.pop(u))
                for fn in extras[u]:
                    fn(last_pv)
                # previous segment's normalization at t=4/6: the reciprocal
                # roundtrip issued at its t=15 has ~4 steps to land, and the
                # early norm opens the proj window for the previous chunk
                if pend_norm is not None and t in (4, 6):
                    pseg, po_sb, prc = pend_norm
                    if pseg == NSEG - 2:
                        emit_norm_nohop(pseg, po_sb, prc, 0 if t == 4 else 1)
                    else:
                        emit_epi_norm(pseg, po_sb, prc, 0 if t == 4 else 1, 2)
                    if t == 6:
                        pend_norm = None
                last_pv = emit_pv(seg, t, e, o_ps)
                if t == NT - 1 and u + 1 < NSTEP:
                    o_sb, rc = emit_epi_stage(seg, o_ps)
                    pend_norm = (seg, o_sb, rc)

            # ---------------- tail: segment B (= seg 8) epilogue ----------
            # after the last exp only ~512 denominators + proj m14/15 remain.
            # No DMA roundtrips: ScalarE (idle now; one table switch) takes
            # the reciprocal straight off the PSUM denominator rows, ONE
            # K=1 matmul broadcasts [1,512] -> [64,512] PSUM, and the ks=1
            # contraction is split K=64+K=64 against a pre-shifted wo copy
            # so the half-1 numerators never need a partition-shift DMA.
            cT, prT, q0B, q1B = SEGS[NSEG - 1]
            wB = q1B - q0B
            # 1/d = exp(-ln d): Ln and Exp live in different table sets, so
            # the order MUST be Ln,Ln,Exp,Exp (pinned) — walrus inserts a
            # ~1.3us ACT_TABLE_LOAD at every Ln<->Exp alternation otherwise.
            # The activations are emitted BEFORE the o_sb staging copies so
            # nothing sits ahead of them in any queue.
            lnv = epi.tile([P, 2, wB], F32, tag="lnv", name="lnv_t")
            rinv = epi.tile([P, 2, wB], BF16, tag="rinv", name="rinv_t")
            prev_act = None
            for half in range(2):
                a = nc.scalar.activation(
                    lnv[HD:HD + 1, half, :], o_ps[half][HD:HD + 1, 0:wB],
                    mybir.ActivationFunctionType.Ln,
                )
                if prev_act is not None:
                    bass._add_dep_helper(a.ins, prev_act.ins, sync=False,
                                         reason="table set order")
                prev_act = a
            for half in range(2):
                a = nc.scalar.activation(
                    rinv[HD:HD + 1, half, :], lnv[HD:HD + 1, half, :],
                    Exp, scale=-1.0,
                )
                bass._add_dep_helper(a.ins, prev_act.ins, sync=False,
                                     reason="table set order")
                prev_act = a
            o_sb = epi.tile([HD + 1, 2, wB], F32, tag=f"osb{wB}",
                            name="osb_tail")
            for half in range(2):
                nc.vector.tensor_copy(o_sb[:, half, :], o_ps[half][:, 0:wB])
            rbs_ps = ps_o.tile([P, QC], F32, tag="o", name="rbs_ps")
            nc.tensor.matmul(
                rbs_ps[0:HD, 0:2 * wB], ones_t[HD:HD + 1, :],
                rinv[HD:HD + 1, :, :], start=True, stop=True,
            )
            # proj ks0 + keepwarms fill the PE while the reciprocal runs
            tail_ps = [ps_st.tile([P, 2, QC], F32, tag="st", name=f"tps{i}")
                       for i in range(2)]
            slots = [tail_ps[0][:, 0, :], tail_ps[0][:, 1, :],
                     tail_ps[1][:, 0, :], tail_ps[1][:, 1, :]]
            for j in range(4):
                m, nn = 14 + j // 2, j % 2
                mm = nc.tensor.matmul(
                    slots[j], at_sb[:, 0, m * P:(m + 1) * P],
                    wo_sb[:, 0, nn * QC:(nn + 1) * QC], start=True, stop=False,
                )
                bass._add_dep_helper(mm.ins, last_pv.ins, sync=False,
                                     reason="tail ks0")
            warm3 = ps_a.tile([P, QC], F32, tag="a", name="warm3")
            for _ in range(55):
                mm = nc.tensor.matmul(
                    warm3[:, 0:P], warm_w, warm_w, start=True, stop=True
                )
                bass._add_dep_helper(mm.ins, last_pv.ins, sync=False,
                                     reason="tail keepwarm")
            # piece-pipelined: normalize 128 at-columns then immediately the
            # proj unit that consumes them; casts/y spread across engines
            stg1 = epi.tile([HD, wB], BF16, tag="stg1", name="stg1_t")
            for p in range(2):
                wp = P
                ss = slice(p * wp, (p + 1) * wp)
                a0 = cT * QC + q0B + p * wp
                m = 14 + p
                nc.vector.tensor_mul(
                    stg1[:, ss], o_sb[0:HD, 1, ss],
                    rbs_ps[0:HD, wB + p * wp:wB + (p + 1) * wp],
                )
                nc.vector.tensor_mul(
                    at_sb[0:HD, prT, a0:a0 + wp], o_sb[0:HD, 0, ss],
                    rbs_ps[0:HD, p * wp:(p + 1) * wp],
                )
                for nn in range(2):
                    j = 2 * p + nn
                    nc.tensor.matmul(
                        slots[j], at_sb[0:HD, 1, m * P:(m + 1) * P],
                        wo_sb[0:HD, 1, nn * QC:(nn + 1) * QC],
                        start=False, stop=False,
                    )
                    nc.tensor.matmul(
                        slots[j], stg1[:, ss],
                        wt_sb[:, nn * QC:(nn + 1) * QC],
                        start=False, stop=True,
                    )
                    ysb = outp.tile([P, QC], BF16, tag="y", name=f"yt_{m}_{nn}")
                    if j % 2 == 0:
                        nc.vector.tensor_copy(ysb[:], slots[j])
                    else:
                        nc.scalar.copy(ysb[:], slots[j])
                    yq = (nc.sync, nc.gpsimd, nc.gpsimd, nc.sync)[j]
                    yq.dma_start(y_r[:, m, nn * QC:(nn + 1) * QC], ysb[:])

    nc.finalize()
    return nc


_NC = None


def _get_nc():
    global _NC
    if _NC is None:
        _NC = build_nc()
    return _NC


def _swiz_w(w):
    # [1024, cols] -> [128, 8, cols]: partition-contiguous for 1-desc rows
    return np.ascontiguousarray(
        w.reshape(KD, P, w.shape[1]).transpose(1, 0, 2)
    )


def _in_maps(x, w_qkv, w_out):
    bf = ml_dtypes.bfloat16
    x = np.asarray(x, dtype=np.float32)
    w_qkv = np.asarray(w_qkv, dtype=np.float32)
    w_out = np.asarray(w_out, dtype=np.float32)
    # xt[p, chunk, k, n'] = x[b].T[k*128+p, chunk*512+n']
    xts = []
    for b in range(2):
        xtb = x[b].T.reshape(KD, P, NQC, QC).transpose(1, 2, 0, 3)
        xts.append(np.ascontiguousarray(xtb).astype(bf))
    wq_g = [_swiz_w(w_qkv[:, 0 * D + g * DG:0 * D + (g + 1) * DG]).astype(bf) for g in range(4)]
    wk_g = [_swiz_w(w_qkv[:, 1 * D + g * DG:1 * D + (g + 1) * DG]).astype(bf) for g in range(4)]
    wv_g = [_swiz_w(w_qkv[:, 2 * D + g * DG:2 * D + (g + 1) * DG]).astype(bf) for g in range(4)]
    wo_g = [
        np.ascontiguousarray(
            w_out[g * DG:(g + 1) * DG, :].reshape(2, P, D).transpose(1, 0, 2)
        ).astype(bf)
        for g in range(4)
    ]
    wt_g = [np.ascontiguousarray(wo_g[g][64:128, 1, :]) for g in range(4)]
    maps = []
    for c in range(8):
        b, g = c // 4, c % 4
        maps.append({
            "xt": xts[b],
            "wq": wq_g[g],
            "wk": wk_g[g],
            "wv": wv_g[g],
            "wo": wo_g[g],
            "wt": wt_g[g],
        })
    return maps


LAST_RESULT = None


def kernel(x, w_qkv, w_out, b_out):
    from concourse.bass_utils import run_bass_kernel_spmd

    nc = _get_nc()
    maps = _in_maps(x, w_qkv, w_out)
    res = run_bass_kernel_spmd(nc, maps, list(range(8)))
    global LAST_RESULT
    LAST_RESULT = res
    out = np.zeros((2, N, D), dtype=np.float32)
    for c in range(8):
        out[c // 4] += np.asarray(res.results[c]["y"], dtype=np.float32)
    out += np.asarray(b_out, dtype=np.float32)[None, None, :]
    return out
